# revision 8
# baseline (speedup 1.0000x reference)
"""Bass/Trainium2 kernel for nn_BipartiteGCNStack (8-core SPMD).

Strategy (sharding_hint): shard A and h_tgt row-wise (n_target) across the 8
cores; layer 1 (src <- tgt) computes per-core partials of P = A^T @ h_tgt and
the column sums of A, combined with a single 4.2MB AllReduce; h_src is then
computed redundantly on every core; layer 2 streams A^T tiles again.

A is streamed from HBM three times per core (64MB per pass) in host-pre-tiled
layouts so every DMA is a contiguous >=2MB read landing directly in matmul
operand layout ([128 contraction partitions, free]):
  atl [4 q][64 c][128 p][512 f]:  atl[q,c,p,f] = A[k*2048 + q*512 + f, c*128+p]
  al  [16 sc][128 p][16 m][512 f]: al[sc,p,m,f] = A[k*2048 + m*128 + p, sc*512+f]

Row/col normalization is folded as per-row scales applied after the matmuls
(exact algebra: row_norm commutes with right-multiplication). BatchNorm
(inference, running stats) is folded into the weights/biases on the host.
Big matmuls run as float32r (single-pass fp32 PE mode, full rate at N>=256).
"""

import sys
import types

sys.path.insert(0, "/opt/trn_rl_repo")

import numpy as np

import concourse.bass as bass  # noqa: F401  (engine namespaces live on nc)
import concourse.mybir as mybir
import concourse.tile as tile
from concourse import bacc
from concourse.bass_utils import run_bass_kernel_spmd
from concourse.masks import make_identity

N_CORES = 8
N_SRC = 8192
N_TGT = 16384
T = N_TGT // N_CORES          # 2048 target rows per core
D_SRC = 256
D_HID = 128
D_OUT = 64
EPS_ROW = 1e-8
EPS_BN = 1e-5

F32 = mybir.dt.float32
F32R = mybir.dt.float32r

USE_F32R = True   # big matmuls in single-pass fp32 mode
AF = F32R if USE_F32R else F32
TRACE = False     # set True (module-level) to profile; exec ns in LAST_EXEC_NS
LAST_EXEC_NS = None

_PROGRAM_CACHE = {}


def _build_program():
    ADD = mybir.AluOpType.add
    MULT = mybir.AluOpType.mult
    RELU = mybir.ActivationFunctionType.Relu

    nc = bacc.Bacc("TRN2", target_bir_lowering=False, debug=False,
                   num_devices=N_CORES)

    atl = nc.dram_tensor("atl", [4, 64, 128, 512], AF, kind="ExternalInput")
    al = nc.dram_tensor("al", [16, 128, 16, 512], AF, kind="ExternalInput")
    hext = nc.dram_tensor("hext", [64, 128, 258], AF, kind="ExternalInput")
    w0f_d = nc.dram_tensor("w0f", [128, 256], F32, kind="ExternalInput")
    b0f_d = nc.dram_tensor("b0f", [1, 128], F32, kind="ExternalInput")
    wb0f_d = nc.dram_tensor("wb0f", [128, 128], F32, kind="ExternalInput")
    bb0f_d = nc.dram_tensor("bb0f", [1, 128], F32, kind="ExternalInput")
    w1f_d = nc.dram_tensor("w1f", [128, 128], AF, kind="ExternalInput")
    b1f_d = nc.dram_tensor("b1f", [128, 1], F32, kind="ExternalInput")
    wout_d = nc.dram_tensor("wout", [128, 64], F32, kind="ExternalInput")
    bout_d = nc.dram_tensor("bout", [1, 64], F32, kind="ExternalInput")

    ones_d = nc.dram_tensor("ones_d", [128, 1], AF, kind="ExternalInput")

    out_d = nc.dram_tensor("out", [T, D_OUT], F32, kind="ExternalOutput")

    with tile.TileContext(nc) as tc:
        with (
            tc.tile_pool(name="const", bufs=1) as constp,
            tc.tile_pool(name="bigio", bufs=1) as bigio,
            tc.tile_pool(name="dram", bufs=1, space="DRAM") as dramp,
        ):
            # ---- constants / params resident in SBUF --------------------
            ident = constp.tile([128, 128], F32, name="ident")
            make_identity(nc, ident)
            ones_col = constp.tile([128, 1], AF, name="ones_col")
            nc.sync.dma_start(ones_col[:], ones_d.ap())

            w0f = constp.tile([128, 256], F32, name="w0f_sb")
            nc.sync.dma_start(w0f[:], w0f_d.ap())
            wb0f = constp.tile([128, 128], F32, name="wb0f_sb")
            nc.sync.dma_start(wb0f[:], wb0f_d.ap())
            w1f = constp.tile([128, 128], AF, name="w1f_sb")
            nc.sync.dma_start(w1f[:], w1f_d.ap())
            wout = constp.tile([128, 64], F32, name="wout_sb")
            nc.sync.dma_start(wout[:], wout_d.ap())
            b1f_c = constp.tile([128, 1], F32, name="b1f_sb")
            nc.sync.dma_start(b1f_c[:], b1f_d.ap())

            def load_bcast(dram_t, n):
                row = constp.tile([1, n], F32, name=f"{dram_t.name}_row")
                nc.sync.dma_start(row[:], dram_t.ap())
                b = constp.tile([128, n], F32, name=f"{dram_t.name}_bc")
                nc.gpsimd.partition_broadcast(b[:], row[:])
                return b

            b0f_b = load_bcast(b0f_d, 128)
            bb0f_b = load_bcast(bb0f_d, 128)
            bout_b = load_bcast(bout_d, 64)

            # long-lived activations
            hT_all = constp.tile([128, T], AF, name="hT_all")       # h_tgt
            rr_row = constp.tile([1, T], F32, name="rr_row")         # 1/rowsum
            hsrc_all = constp.tile([128, N_SRC], AF, name="hsrc_all")

            ar_in = dramp.tile([129, N_SRC], F32, name="ar_in")
            ar_out = dramp.tile([129, N_SRC], F32, name="ar_out",
                                addr_space="Shared")

            # =============== PASS 1: layer 0 (tgt <- src) ===============
            with (
                tc.tile_pool(name="hextp", bufs=1) as hextp,
                tc.tile_pool(name="p1w", bufs=1) as p1w,
                tc.tile_pool(name="ps1", bufs=1, space="PSUM") as ps1,
            ):
                hx = hextp.tile([128, 64 * 258], AF, name="hx")
                nc.sync.dma_start(hx[:].rearrange("p (c f) -> p c f", c=64),
                                  hext.ap().rearrange("c p f -> p c f"))

                for q in range(4):
                    m0 = [ps1.tile([128, 258], F32, name=f"m0_{q}_{t}",
                                   tag=f"m0_{t}", bufs=1) for t in range(4)]
                    for g in range(8):  # 8 source-chunk groups of 8
                        at8 = bigio.tile([128, 8 * 512], AF, name=f"at_{q}_{g}",
                                         tag="big", bufs=3)
                        nc.sync.dma_start(
                            at8[:].rearrange("p (c f) -> p c f", c=8),
                            atl.ap()[q, g * 8:(g + 1) * 8].rearrange(
                                "c p f -> p c f"))
                        for ci in range(8):
                            c = g * 8 + ci
                            for t in range(4):
                                nc.tensor.matmul(
                                    m0[t][:],
                                    lhsT=at8[:, ci * 512 + t * 128:
                                                ci * 512 + (t + 1) * 128],
                                    rhs=hx[:, c * 258:(c + 1) * 258],
                                    start=(c == 0), stop=(c == 63))
                    # epilogue: rowsum -> recip, scale, transpose, @W0f, relu
                    for t in range(4):
                        m = q * 4 + t
                        rs = p1w.tile([128, 1], F32, name=f"rs{m}", tag="rs",
                                      bufs=2)
                        nc.vector.tensor_scalar_max(rs[:], m0[t][:, 256:257],
                                                    EPS_ROW)
                        rr = p1w.tile([128, 1], F32, name=f"rr{m}", tag="rr",
                                      bufs=2)
                        nc.vector.reciprocal(rr[:], rs[:])
                        rrt = ps1.tile([1, 128], F32, name=f"rrt{m}", tag="tp",
                                       bufs=1)
                        nc.tensor.transpose(rrt[:], rr[:], ident[:])
                        nc.vector.tensor_copy(rr_row[0:1, m * 128:(m + 1) * 128],
                                              rrt[:])
                        m0n = p1w.tile([128, 256], F32, name=f"m0n{m}",
                                       tag="m0n", bufs=2)
                        nc.vector.tensor_scalar_mul(m0n[:], m0[t][:, 0:256],
                                                    rr[:])
                        m0nT = p1w.tile([128, 256], F32, name=f"m0nT{m}",
                                        tag="m0nT", bufs=2)
                        for i in range(2):
                            tp = ps1.tile([128, 128], F32, name=f"tp{m}_{i}",
                                          tag="tp", bufs=1)
                            nc.tensor.transpose(
                                tp[:], m0n[:, i * 128:(i + 1) * 128], ident[:])
                            nc.vector.tensor_copy(
                                m0nT[:, i * 128:(i + 1) * 128], tp[:])
                        hpre = ps1.tile([128, 128], F32, name=f"hpre{m}",
                                        tag="hpre", bufs=2)
                        for i in range(2):
                            nc.tensor.matmul(
                                hpre[:],
                                lhsT=m0nT[:, i * 128:(i + 1) * 128],
                                rhs=w0f[:, i * 128:(i + 1) * 128],
                                start=(i == 0), stop=(i == 1))
                        htmp = p1w.tile([128, 128], F32, name=f"htmp{m}",
                                        tag="htmp", bufs=2)
                        nc.vector.tensor_tensor(htmp[:], hpre[:], b0f_b[:],
                                                op=ADD)
                        nc.scalar.activation(
                            hT_all[:, m * 128:(m + 1) * 128], htmp[:], RELU)

            # ====== PASS 2: P^T = h_tgt^T @ A (partial) + colsum ========
            with (
                tc.tile_pool(name="p2w", bufs=1) as p2w,
                tc.tile_pool(name="ps2", bufs=1, space="PSUM") as ps2,
            ):
                for sc in range(16):
                    pp = ps2.tile([128, 512], F32, name=f"pp{sc}", tag="pp",
                                  bufs=2)
                    cs = ps2.tile([1, 512], F32, name=f"cs{sc}", tag="cs",
                                  bufs=2)
                    for h in range(2):
                        a8 = bigio.tile([128, 8 * 512], AF, name=f"a2_{sc}_{h}",
                                        tag="big", bufs=3)
                        nc.sync.dma_start(
                            a8[:].rearrange("p (m f) -> p m f", m=8),
                            al.ap()[sc, :, h * 8:(h + 1) * 8])
                        for mi in range(8):
                            m = h * 8 + mi
                            nc.tensor.matmul(
                                pp[:],
                                lhsT=hT_all[:, m * 128:(m + 1) * 128],
                                rhs=a8[:, mi * 512:(mi + 1) * 512],
                                start=(m == 0), stop=(m == 15))
                            nc.tensor.matmul(
                                cs[:], lhsT=ones_col[:],
                                rhs=a8[:, mi * 512:(mi + 1) * 512],
                                start=(m == 0), stop=(m == 15))
                    st = p2w.tile([128, 512], F32, name=f"st{sc}", tag="st",
                                  bufs=3)
                    nc.vector.tensor_copy(st[:], pp[:])
                    nc.sync.dma_start(ar_in[0:128, sc * 512:(sc + 1) * 512],
                                      st[:])
                    st2 = p2w.tile([1, 512], F32, name=f"st2{sc}", tag="st2",
                                   bufs=3)
                    nc.vector.tensor_copy(st2[:], cs[:])
                    nc.sync.dma_start(ar_in[128:129, sc * 512:(sc + 1) * 512],
                                      st2[:])

            # =============== AllReduce over the 8 cores =================
            nc.gpsimd.collective_compute(
                "AllReduce", mybir.AluOpType.add,
                replica_groups=[list(range(N_CORES))],
                ins=[ar_in.opt()], outs=[ar_out.opt()])

            # ====== h_src = relu((P @ Wb0f) * (1/colsum) + bb0f) ========
            with (
                tc.tile_pool(name="hsw", bufs=1) as hsw,
                tc.tile_pool(name="ps3", bufs=1, space="PSUM") as ps3,
            ):
                ptT = hsw.tile([128, N_SRC], F32, name="ptT")
                nc.sync.dma_start(ptT[:], ar_out[0:128, :])
                cs64 = hsw.tile([64, 128], F32, name="cs64")
                nc.sync.dma_start(
                    cs64[:], ar_out[128:129, :].rearrange("o (c f) -> (o c) f",
                                                          c=64))
                cst = ps3.tile([128, 64], F32, name="cst", tag="cst", bufs=1)
                nc.tensor.transpose(cst[:], cs64[:], ident[0:64, 0:64])
                csq = hsw.tile([128, 64], F32, name="csq")
                nc.vector.tensor_scalar_max(csq[:], cst[:], EPS_ROW)
                rcq = hsw.tile([128, 64], F32, name="rcq")
                nc.vector.reciprocal(rcq[:], csq[:])

                for c in range(64):
                    hs = ps3.tile([128, 128], F32, name=f"hs{c}", tag="hs",
                                  bufs=2)
                    nc.tensor.matmul(hs[:],
                                     lhsT=ptT[:, c * 128:(c + 1) * 128],
                                     rhs=wb0f[:], start=True, stop=True)
                    hsc = hsw.tile([128, 128], F32, name=f"hsc{c}", tag="hsc",
                                   bufs=3)
                    nc.vector.tensor_scalar_mul(hsc[:], hs[:], rcq[:, c:c + 1])
                    hsb = hsw.tile([128, 128], F32, name=f"hsb{c}", tag="hsb",
                                   bufs=3)
                    nc.vector.tensor_tensor(hsb[:], hsc[:], bb0f_b[:], op=ADD)
                    nc.scalar.activation(hsrc_all[:, c * 128:(c + 1) * 128],
                                         hsb[:], RELU)

            # ========== PASS 3: layer 2 (tgt <- src) + output ===========
            with (
                tc.tile_pool(name="p3w", bufs=1) as p3w,
                tc.tile_pool(name="ps4", bufs=1, space="PSUM") as ps4,
            ):
                for q in range(4):
                    m2 = ps4.tile([128, 512], F32, name=f"m2_{q}", tag="m2",
                                  bufs=2)
                    for g in range(8):
                        at8 = bigio.tile([128, 8 * 512], AF, name=f"at3_{q}_{g}",
                                         tag="big", bufs=3)
                        nc.sync.dma_start(
                            at8[:].rearrange("p (c f) -> p c f", c=8),
                            atl.ap()[q, g * 8:(g + 1) * 8].rearrange(
                                "c p f -> p c f"))
                        for ci in range(8):
                            c = g * 8 + ci
                            nc.tensor.matmul(
                                m2[:],
                                lhsT=hsrc_all[:, c * 128:(c + 1) * 128],
                                rhs=at8[:, ci * 512:(ci + 1) * 512],
                                start=(c == 0), stop=(c == 63))
                    # epilogue (transposed space: d on partitions)
                    rrb = p3w.tile([128, 512], F32, name=f"rrb{q}", tag="rrb",
                                   bufs=2)
                    nc.gpsimd.partition_broadcast(
                        rrb[:], rr_row[0:1, q * 512:(q + 1) * 512])
                    x2 = p3w.tile([128, 512], AF, name=f"x2{q}", tag="x2",
                                  bufs=2)
                    nc.vector.tensor_tensor(x2[:], m2[:], rrb[:], op=MULT)
                    h2 = ps4.tile([128, 512], F32, name=f"h2{q}", tag="h2",
                                  bufs=2)
                    nc.tensor.matmul(h2[:], lhsT=w1f[:], rhs=x2[:],
                                     start=True, stop=True)
                    h2T = p3w.tile([128, 512], F32, name=f"h2T{q}", tag="h2T",
                                   bufs=2)
                    nc.scalar.activation(h2T[:], h2[:], RELU, bias=b1f_c[:])
                    outst = p3w.tile([128, 256], F32, name=f"outst{q}",
                                     tag="outst", bufs=2)
                    for t in range(4):
                        ot = ps4.tile([128, 64], F32, name=f"ot{q}_{t}",
                                      tag="ot", bufs=2)
                        nc.tensor.matmul(ot[:],
                                         lhsT=h2T[:, t * 128:(t + 1) * 128],
                                         rhs=wout[:], start=True, stop=True)
                        nc.vector.tensor_tensor(outst[:, t * 64:(t + 1) * 64],
                                                ot[:], bout_b[:], op=ADD)
                    nc.sync.dma_start(
                        out_d.ap().rearrange("(q t p) j -> q p t j",
                                             t=4, p=128)[q],
                        outst[:].rearrange("p (t j) -> p t j", t=4))

    nc.compile()
    return nc


def _prep_host(inputs):
    f = np.float32
    A = np.ascontiguousarray(np.asarray(inputs["A"], dtype=f))
    H = np.ascontiguousarray(np.asarray(inputs["H_source"], dtype=f))
    AT = np.ascontiguousarray(A.T)  # [N_SRC, N_TGT]

    hext = np.concatenate([H, np.ones((N_SRC, 1), f),
                           np.zeros((N_SRC, 1), f)], axis=1)
    hext = np.ascontiguousarray(hext.reshape(64, 128, 258))

    def fold(W, b, gamma, beta, mean, var):
        sc = (gamma / np.sqrt(var + EPS_BN)).astype(f)
        Wf = (W * sc[None, :]).astype(f)
        bf = ((b - mean) * sc + beta).astype(f)
        return Wf, bf

    W0f, b0f = fold(np.asarray(inputs["W0"], f), np.asarray(inputs["b0"], f),
                    np.asarray(inputs["bn_f_gamma"], f)[0],
                    np.asarray(inputs["bn_f_beta"], f)[0],
                    np.asarray(inputs["bn_f_mean"], f)[0],
                    np.asarray(inputs["bn_f_var"], f)[0])
    Wb0f, bb0f = fold(np.asarray(inputs["Wb0"], f), np.asarray(inputs["bb0"], f),
                      np.asarray(inputs["bn_b_gamma"], f),
                      np.asarray(inputs["bn_b_beta"], f),
                      np.asarray(inputs["bn_b_mean"], f),
                      np.asarray(inputs["bn_b_var"], f))
    W1f, b1f = fold(np.asarray(inputs["W1"], f), np.asarray(inputs["b1"], f),
                    np.asarray(inputs["bn_f_gamma"], f)[1],
                    np.asarray(inputs["bn_f_beta"], f)[1],
                    np.asarray(inputs["bn_f_mean"], f)[1],
                    np.asarray(inputs["bn_f_var"], f)[1])

    shared = {
        "hext": hext,
        "ones_d": np.ones((128, 1), f),
        "w0f": np.ascontiguousarray(
            W0f.reshape(2, 128, 128).transpose(1, 0, 2).reshape(128, 256)),
        "b0f": b0f.reshape(1, 128).copy(),
        "wb0f": np.ascontiguousarray(Wb0f),
        "bb0f": bb0f.reshape(1, 128).copy(),
        "w1f": np.ascontiguousarray(W1f),
        "b1f": b1f.reshape(128, 1).copy(),
        "wout": np.ascontiguousarray(np.asarray(inputs["Wout"], f)),
        "bout": np.asarray(inputs["bout"], f).reshape(1, 64).copy(),
    }

    in_maps = []
    for k in range(N_CORES):
        Ak = A[k * T:(k + 1) * T]                 # [2048, 8192]
        ATk = AT[:, k * T:(k + 1) * T]            # [8192, 2048] view
        atl_k = np.ascontiguousarray(
            ATk.reshape(64, 128, 4, 512).transpose(2, 0, 1, 3))
        al_k = np.ascontiguousarray(
            Ak.reshape(16, 128, 16, 512).transpose(2, 1, 0, 3))
        in_maps.append({"atl": atl_k, "al": al_k, **shared})
    return in_maps


def _install_trace_hook():
    try:
        import antenv
        from trn_agent_boot.trn_boot import _ntff_profile_via_ctypes
        hooks_mod = types.ModuleType("antenv.axon_hooks")
        _hook = _ntff_profile_via_ctypes("/opt/axon/libaxon_pjrt.so")
        hooks_mod.get_axon_ntff_profile_hook = lambda: _hook
        hooks_mod.set_axon_ntff_profile_hook = lambda h: None
        sys.modules["antenv.axon_hooks"] = hooks_mod
        antenv.axon_hooks = hooks_mod
        return True
    except Exception:
        return False


def kernel(**inputs):
    global LAST_EXEC_NS
    if "prog" not in _PROGRAM_CACHE:
        _PROGRAM_CACHE["prog"] = _build_program()
    nc = _PROGRAM_CACHE["prog"]
    in_maps = _prep_host(inputs)
    kwargs = {}
    if TRACE and _install_trace_hook():
        kwargs["trace"] = True
    res = run_bass_kernel_spmd(nc, in_maps, core_ids=list(range(N_CORES)),
                               **kwargs)
    LAST_EXEC_NS = res.exec_time_ns
    out = np.concatenate([res.results[k]["out"] for k in range(N_CORES)],
                         axis=0)
    return out.astype(np.float32)
